# revision 14
# baseline (speedup 1.0000x reference)
"""Trainium2 Bass kernel for nn_Head (single-head causal attention).

Contract: kernel(**inputs) takes FULL inputs (x [8,2048,1024] f32,
Wk/Wq/Wv [64,1024] f32) and returns the FULL output [8,2048,64] f32.
Data-parallel over batch B=8 across the 8 NeuronCores (one batch row per
core); each core runs an identical single-core program.

v5 design (strip-pipelined, row-tiled, PSUM-direct exp):
  - Host packs inputs in SBUF layout so every DMA moves 4-8KB contiguous
    lines per partition (DMA descriptor-issue cost is per line; 1KB-line
    patterns cost ~2.7us per strip to issue, 8KB lines ~0.4us):
      xh  [4, 128, 8, 512]  xT strips  (strip-major)
      wh  [128, 2, 8, 128]  wts[0]=[Wk/32;Wq], wts[1]=[Wv;Wk/32]
      ch  [128, 1024] u8    constants: identities (both partition halves),
                            tri mask, 65x65 fp32 identity
    All input DMAs ride ONE ring (sync) so transfers complete strictly in
    strip order at full bandwidth.
  - Per 512-column strip: kq proj (kT -> kk2 top, qT stays at partitions
    64:128) -> q-even tiles moved to partitions 0:64 by a PE
    double-transpose (NO DMA: the DMA engines are saturated by the input
    stream, a 32KB SBUF->SBUF copy was completing ~5us late) -> vk proj
    (vT + the second kT copy -> kk2 bottom; the v projection would
    otherwise idle half the PE columns) -> v transposes -> ST row-tiled
    pairs: tile 2m (PE rows 0:63) and tile 2m+1 (rows 64:127) run in
    LOCKSTEP because both rhs streams read the same kk2 tile/columns ->
    ONE exp per pair straight from the [128,2,512] PSUM group (ACT reads
    PSUM at full rate; DVE at ~half rate) -> tri-mask diagonal blocks
    (DVE, bf16) -> PV of the previous strip interleaved between ST pairs
    (row-tiled K=64 pairs into OT_A/OT_B; vaug ones-column gives the
    softmax denominator) -> epilogue: OT_A+OT_B, PE transposes,
    reciprocal+normalize on DVE, per-strip DMA out (gpsimd issue).
  - HAM: the PE re-throttles to 1.2 GHz after idle windows and re-warms
    only after ~3.4us sustained activity.  Dummy matmuls run while the
    input DMA lands (their PSUM is read once by a DVE copy so dead-code
    elimination keeps them); the schedule keeps the PE dense after.
  - A primer activation pulls the ~1.3us exp-table load into the DMA
    shadow.
"""

import sys

if "/opt/trn_rl_repo" not in sys.path:
    sys.path.insert(0, "/opt/trn_rl_repo")

import numpy as np

B = 8
T = 2048
C = 1024
H = 64
P = 128
CB = C // P        # 8 contraction chunks of 128
TJ = T // 512      # 4 column strips of 512
NT = T // P        # 16 s-tiles
N_CORES = 8

PIECES = [4 * j + 4 for j in range(TJ)]   # ST pieces per strip: 4, 8, 12, 16
N_DUMMY = 5                               # PE warm-up matmuls (N=512)

_NC_CACHE = {}


def _build_nc():
    import concourse.bass as bass
    import concourse.mybir as mybir
    import concourse.tile as tile
    from concourse.bass import ts

    fp32 = mybir.dt.float32
    bf16 = mybir.dt.bfloat16
    u8 = mybir.dt.uint8
    EXP = mybir.ActivationFunctionType.Exp
    MULT = mybir.AluOpType.mult
    ADD = mybir.AluOpType.add

    nc = bass.Bass(target_bir_lowering=False, debug=False)
    xh_d = nc.declare_dram_parameter("xh", [TJ, P, CB, 512], bf16, isOutput=False)
    wh_d = nc.declare_dram_parameter("wh", [P, 2, CB, P], bf16, isOutput=False)
    ch_d = nc.declare_dram_parameter("ch", [P, 1024], u8, isOutput=False)
    out_d = nc.declare_dram_parameter("out", [T, H], fp32, isOutput=True)

    from contextlib import ExitStack

    with tile.TileContext(nc) as tc, ExitStack() as stk:
        pers = stk.enter_context(tc.tile_pool(name="pers", bufs=1))
        xt0a = pers.tile([P, CB // 2, 512], bf16, tag="xt0a", name="xt0a")
        xt0b = pers.tile([P, CB // 2, 512], bf16, tag="xt0b", name="xt0b")
        xts = [None] + [
            pers.tile([P, CB, 512], bf16, tag=f"xt{j}", name=f"xt{j}")
            for j in range(1, TJ)
        ]
        w_sb = pers.tile([P, 2, CB, P], bf16, tag="w_sb", name="w_sb")
        c_sb = pers.tile([P, 1024], u8, tag="c_sb", name="c_sb")
        kqt = [pers.tile([P, 512], bf16, tag=f"kqt{j}", name=f"kqt{j}") for j in range(TJ)]
        kk2 = [pers.tile([P, 512], bf16, tag=f"kk2{j}", name=f"kk2{j}") for j in range(TJ)]
        vk = [pers.tile([P, 512], bf16, tag=f"vk{j}", name=f"vk{j}") for j in range(TJ)]
        qdup = [pers.tile([P, 512], bf16, tag=f"qd{j}", name=f"qd{j}") for j in range(TJ)]
        qnat = [pers.tile([P, 2, H], bf16, tag=f"qn{j}", name=f"qn{j}") for j in range(TJ)]
        pt_sb = [pers.tile([P, PIECES[j], 512], bf16, tag=f"pt{j}", name=f"pt{j}") for j in range(TJ)]
        vaug = [pers.tile([P, 4, H + 1], bf16, tag=f"va{j}", name=f"va{j}") for j in range(TJ)]
        oadd = [pers.tile([H + 1, 512], fp32, tag=f"oa{j}", name=f"oa{j}") for j in range(TJ)]
        o_sb = [pers.tile([P, 4, H], fp32, tag=f"o{j}", name=f"o{j}") for j in range(TJ)]
        rec = [pers.tile([P, 4], fp32, tag=f"rc{j}", name=f"rc{j}") for j in range(TJ)]
        scr_w = pers.tile([P, P], bf16, tag="scr_w", name="scr_w")
        scr_x = pers.tile([P, 512], bf16, tag="scr_x", name="scr_x")
        scr_rd = pers.tile([P, 1], fp32, tag="scr_rd", name="scr_rd")
        prim = pers.tile([P, 1], fp32, tag="prim", name="prim")
        prim_o = pers.tile([P, 1], fp32, tag="prim_o", name="prim_o")

        # constant views (shipped via DMA in ch)
        identb_lo = c_sb[0:H, 0:128].bitcast(bf16)        # [64, 64] @ 0:64
        identb_hi = c_sb[H:P, 0:128].bitcast(bf16)        # [64, 64] @ 64:128
        identb128 = c_sb[:, 128:384].bitcast(bf16)        # [128, 128]
        tri = c_sb[:, 384:640].bitcast(bf16)              # [128, 128]
        ident65 = c_sb[0 : H + 1, 640:900].bitcast(fp32)  # [65, 65]

        # ---- input DMAs first: ONE ring (sync), strict strip order ----
        nc.sync.dma_start(w_sb[:], wh_d[:])
        nc.sync.dma_start(c_sb[:], ch_d[:])
        nc.sync.dma_start(xt0a[:], xh_d[0, :, 0 : CB // 2, :])
        nc.sync.dma_start(xt0b[:], xh_d[0, :, CB // 2 : CB, :])
        for j in range(1, TJ):
            nc.sync.dma_start(xts[j][:], xh_d[j])

        # ---- early gpsimd work: scratch memsets ----
        nc.gpsimd.memset(scr_w[:], 0.0)
        nc.gpsimd.memset(scr_x[:], 0.0)
        nc.gpsimd.memset(prim[:], 0.0)
        for j in range(TJ):
            nc.gpsimd.memset(vaug[j][:, :, H], 1.0)

        # ---- scalar primer: pull the exp table load into the DMA shadow ----
        nc.scalar.activation(prim_o[:], prim[:], EXP)

        def xsrc(j, cb):
            if j == 0:
                half = xt0a if cb < CB // 2 else xt0b
                return half[:, cb % (CB // 2), :]
            return xts[j][:, cb, :]

        # ---- PE warm-up dummies (read once by DVE so DCE keeps them) ----
        with tc.tile_pool(name="scrp", bufs=1, space="PSUM") as scrp:
            scr_ps = scrp.tile([P, 512], fp32, tag="scr", name="scr_ps")
            for k in range(N_DUMMY):
                nc.tensor.matmul(
                    scr_ps, scr_w[:], scr_x[:], start=(k == 0), stop=(k == N_DUMMY - 1)
                )
            nc.vector.tensor_copy(scr_rd[:], scr_ps[:, 0:1])

        with (
            tc.tile_pool(name="prjp", bufs=2, space="PSUM") as prjp,   # 2 banks
            tc.tile_pool(name="stp", bufs=2, space="PSUM") as stp,     # 4 banks
            tc.tile_pool(name="otp", bufs=1, space="PSUM") as otp,     # 2 banks
        ):
            ot_a = otp.tile([H + 1, 512], fp32, tag="ota", name="ot_a")
            ot_b = otp.tile([H + 1, 512], fp32, tag="otb", name="ot_b")

            def emit_kqp(j):
                kq_ps = prjp.tile([P, 512], fp32, tag="prj", name=f"kq{j}")
                for cb in range(CB):
                    nc.tensor.matmul(
                        kq_ps, w_sb[:, 0, cb, :], xsrc(j, cb),
                        start=(cb == 0), stop=(cb == CB - 1),
                    )
                # kT -> kk2 top (A-side rhs); qT -> kqt bottom (B weights).
                nc.vector.tensor_copy(kk2[j][0:H, :], kq_ps[0:H, :])
                nc.vector.tensor_copy(kqt[j][H:P, :], kq_ps[H:P, :])

            def emit_qdup(j):
                # q EVEN tiles (4j, 4j+2) to partitions 0:64 via PE
                # double-transpose (no DMA - the rings are busy with xt).
                for m in range(2):
                    blk = 2 * m                      # strip-local tile 0 / 2
                    t1 = prjp.tile([P, H], bf16, tag="prj", name=f"q1_{j}_{m}")
                    nc.tensor.transpose(t1, kqt[j][H:P, ts(blk, P)], identb_hi)
                    nc.vector.tensor_copy(qnat[j][:, m, :], t1)
                for m in range(2):
                    blk = 2 * m
                    t2 = prjp.tile([H, P], bf16, tag="prj", name=f"q2_{j}_{m}")
                    nc.tensor.transpose(t2, qnat[j][:, m, :], identb128)
                    nc.vector.tensor_copy(qdup[j][0:H, ts(blk, P)], t2)

            def emit_vkp(j):
                v_ps = prjp.tile([P, 512], fp32, tag="prj", name=f"v{j}")
                for cb in range(CB):
                    nc.tensor.matmul(
                        v_ps, w_sb[:, 1, cb, :], xsrc(j, cb),
                        start=(cb == 0), stop=(cb == CB - 1),
                    )
                nc.vector.tensor_copy(vk[j][0:H, :], v_ps[0:H, :])
                nc.vector.tensor_copy(kk2[j][H:P, :], v_ps[H:P, :])
                # v transposes: vk[j][0:64, block m] -> vaug[j][:, m, 0:64]
                for m in range(4):
                    vps = prjp.tile([P, H], bf16, tag="prj", name=f"vt{j}_{m}")
                    nc.tensor.transpose(vps, vk[j][0:H, ts(m, P)], identb_lo)
                    nc.vector.tensor_copy(vaug[j][:, m, 0:H], vps)

            def emit_st(j):
                # row-tiled pairs in lockstep: tile 2m on PE rows 0:63,
                # tile 2m+1 on rows 64:127, both streaming kk2[j] columns.
                for m in range(2 * j + 2):
                    iA, iB = 2 * m, 2 * m + 1
                    o = max(0, P * iA - 512 * j)
                    s2 = stp.tile([P, 2, 512], fp32, tag="st", name=f"s{j}_{m}")
                    nc.tensor.matmul(
                        s2[:, 0, o:512],
                        qdup[iA // 4][0:H, ts(iA % 4, P)],
                        kk2[j][0:H, o:512],
                        start=True, stop=True,
                    )
                    nc.tensor.matmul(
                        s2[:, 1, o:512],
                        kqt[iB // 4][H:P, ts(iB % 4, P)],
                        kk2[j][H:P, o:512],
                        start=True, stop=True,
                    )
                    nc.scalar.activation(
                        pt_sb[j][:, 2 * m : 2 * m + 2, o:512],
                        s2[:, :, o:512],
                        EXP,
                    )
                    yield m

            def emit_tri(j):
                # tri-mask the 4 diagonal pieces of strip j (before PV j)
                for i in range(4 * j, 4 * j + 4):
                    o = P * i - 512 * j
                    nc.vector.tensor_tensor(
                        pt_sb[j][:, i, o : o + P],
                        pt_sb[j][:, i, o : o + P],
                        tri, MULT,
                    )

            def emit_pv(j, lo=0, hi=None):
                n = PIECES[j]
                if hi is None:
                    hi = n
                for i in range(lo, hi):
                    o = max(0, P * i - 512 * j)
                    nc.tensor.matmul(
                        ot_a[:, o:512],
                        vaug[i // 4][0:H, i % 4, :],
                        pt_sb[j][0:H, i, o:512],
                        start=(i == 0), stop=(i == n - 1),
                    )
                    nc.tensor.matmul(
                        ot_b[:, o:512],
                        vaug[i // 4][H:P, i % 4, :],
                        pt_sb[j][H:P, i, o:512],
                        start=(i == 0), stop=(i == n - 1),
                    )

            def emit_epilogue(j):
                # OT = A + B into SBUF (one PSUM operand per DVE op)
                nc.vector.tensor_copy(oadd[j][:], ot_a[:, :])
                nc.vector.tensor_tensor(oadd[j][:], ot_b[:, :], oadd[j][:], ADD)
                for m in range(4):
                    ops = prjp.tile([P, H + 1], fp32, tag="prj", name=f"or{j}_{m}")
                    nc.tensor.transpose(ops, oadd[j][:, ts(m, P)], ident65)
                    nc.vector.reciprocal(rec[j][:, m : m + 1], ops[:, H : H + 1])
                    nc.vector.tensor_scalar_mul(
                        o_sb[j][:, m, :], ops[:, 0:H], rec[j][:, m : m + 1]
                    )
                nc.gpsimd.dma_start(
                    out_d[ts(j, 512), :].rearrange("(m p) d -> p m d", p=P),
                    o_sb[j][:],
                )

            # ---- pipeline: PV lags exp by one strip, interleaved between
            # the next strip's ST pairs so the PE stays fed ----
            for j in range(TJ):
                emit_kqp(j)
                emit_qdup(j)
                emit_vkp(j)
                if j > 0:
                    emit_tri(j - 1)
                npv = PIECES[j - 1] if j > 0 else 0
                done = 0
                nst = 2 * j + 2
                for m in emit_st(j):
                    if j > 0:
                        take = (npv * (m + 1)) // nst
                        emit_pv(j - 1, done, take)
                        done = take
                if j > 0:
                    emit_pv(j - 1, done, npv)
                    emit_epilogue(j - 1)
            emit_tri(TJ - 1)
            emit_pv(TJ - 1)
            emit_epilogue(TJ - 1)

    return nc


def _split_multiwaits(nc):
    """Walrus codegen only supports one sync-wait command per instruction;
    hoist extra waits onto NoOps inserted just before (same engine queue,
    identical semantics since engines execute their queue in order)."""
    import concourse.mybir as mybir

    n = 0
    for fn in nc.m.functions:
        for block in fn.blocks:
            new_insts = []
            for inst in block.instructions:
                si = inst.sync_info
                if si is not None and si.on_wait and len(si.on_wait) > 1:
                    waits = list(si.on_wait)
                    for w in waits[:-1]:
                        n += 1
                        new_insts.append(
                            mybir.InstNoOp(
                                name=f"WH-{n}", engine=inst.engine, ins=[], outs=[],
                                sync_info=mybir.SyncInfo(on_wait=[w], on_update=[]),
                            )
                        )
                    si.on_wait = waits[-1:]
                new_insts.append(inst)
            block.instructions = new_insts
    return nc


def _get_nc():
    if "nc" not in _NC_CACHE:
        _NC_CACHE["nc"] = _split_multiwaits(_build_nc())
    return _NC_CACHE["nc"]


def _make_consts():
    import ml_dtypes

    bf16 = ml_dtypes.bfloat16
    ch = np.zeros((P, 1024), dtype=np.uint8)
    idb2 = np.zeros((P, H), dtype=bf16)
    idb2[0:H] = np.eye(H, dtype=bf16)
    idb2[H:P] = np.eye(H, dtype=bf16)
    ch[:, 0:128] = idb2.view(np.uint8)
    ch[:, 128:384] = np.eye(P, dtype=bf16).view(np.uint8)
    tri = np.triu(np.ones((P, P), dtype=np.float32)).astype(bf16)
    ch[:, 384:640] = tri.view(np.uint8)
    id65 = np.eye(H + 1, dtype=np.float32)
    ch[0 : H + 1, 640:900] = id65.view(np.uint8)
    return ch


def _make_in_maps(x, Wk, Wq, Wv):
    import ml_dtypes

    bf16 = ml_dtypes.bfloat16
    scale = 1.0 / np.sqrt(np.float32(C))
    wkq = np.concatenate([Wk * scale, Wq], axis=0).T  # [C, 128]
    wvk = np.concatenate([Wv, Wk * scale], axis=0).T  # [C, 128]
    # [2, C, 128] -> [p, w, cb, m]
    wh = np.ascontiguousarray(
        np.stack([wkq, wvk]).astype(bf16)
        .reshape(2, CB, P, P).transpose(2, 0, 1, 3)
    )
    ch = _make_consts()
    in_maps = []
    for b in range(B):
        xt = x[b].T.astype(bf16)  # [C, T]
        # [cb*128+p, j*512+t] -> [j, p, cb, t]
        xhb = np.ascontiguousarray(
            xt.reshape(CB, P, TJ, 512).transpose(2, 1, 0, 3)
        )
        in_maps.append({"xh": xhb, "wh": wh, "ch": ch})
    return in_maps


def run(x, Wk, Wq, Wv, trace=False):
    from concourse.bass_utils import run_bass_kernel_spmd

    nc = _get_nc()
    in_maps = _make_in_maps(x, Wk, Wq, Wv)
    res = run_bass_kernel_spmd(nc, in_maps, core_ids=list(range(N_CORES)), trace=trace)
    out = np.stack([np.asarray(res.results[b]["out"]) for b in range(B)], axis=0)
    return out.astype(np.float32), res


def kernel(x, Wk, Wq, Wv):
    out, _ = run(x, Wk, Wq, Wv, trace=False)
    return out


# revision 15
# speedup vs baseline: 1.1495x; 1.1495x over previous
"""Trainium2 Bass kernel for nn_Head (single-head causal attention).

Contract: kernel(**inputs) takes FULL inputs (x [8,2048,1024] f32,
Wk/Wq/Wv [64,1024] f32) and returns the FULL output [8,2048,64] f32.
Data-parallel over batch B=8 across the 8 NeuronCores (one batch row per
core); each core runs an identical single-core program.

v6 design (strip-pipelined, PSUM-direct exp, proj interleaved):
  - Host packs inputs in SBUF layout so every DMA moves 4-8KB contiguous
    lines per partition (DMA descriptor issue cost is per line: 1KB-line
    patterns took ~2.7us per strip to issue, 8KB lines ~0.6us):
      xh  [4, 128, 8, 512]  xT strips (strip-major)
      wh  [128, 2, 8, 128]  w[0]=[Wv;Wq], w[1]=[Wk/32;Wk/32]
      ch  [128, 1024] u8    constants (identities, tri mask)
    All input DMAs ride ONE ring (sync) so transfers complete strictly in
    strip order at full bandwidth.
  - Projections per strip: P1=[Wv;Wq] -> vT @ partitions 0:64 + qT @
    64:128; P2=[Wk/32;Wk/32] -> kT(scaled) @ 64:128 (kk2).  Everything
    the attention loop needs then lives at partitions 64:128 (q weights,
    kT rhs) with NO cross-partition moves (DVE/ACT/GpSimd are lane-wise;
    only PE transposes or DMA can cross, and both proved expensive).
  - ST: s-tile i as a K=64 matmul on the upper PE half (q_i stationary,
    kk2 streaming).  Two consecutive tiles land in one [128,2,512]
    two-bank PSUM group -> ONE exp per group DIRECTLY from PSUM (ACT
    reads PSUM at full rate, DVE at ~half rate) -> tri-mask diagonal
    blocks on DVE (bf16) -> PV row-tiled pairs: vaug/pt upper and lower
    halves run in LOCKSTEP (same tile, same columns) into OT_A/OT_B;
    the vaug ones-column accumulates the softmax denominator.
  - Schedule: the ST/exp phase of strip j is ACT-paced; the projections
    of strip j+1 and the PV of strip j-1 are interleaved BETWEEN ST
    groups so the PE never idles (idle fragments re-throttle the PE to
    1.2 GHz via HAM; it only re-warms after ~3.4us of sustained work).
  - Epilogue per strip: OT_A+OT_B -> SBUF, 4 PE transposes into one
    PSUM bank, reciprocal of the denominator + normalize on DVE,
    per-strip DMA out (issued from gpsimd).
  - Dummy matmuls cover the input-DMA latency so real work starts with
    the PE already warm (their PSUM is read once by a DVE copy so DCE
    keeps them).  A primer activation pulls the ~1.3us exp-table load
    into the DMA shadow.
"""

import sys

if "/opt/trn_rl_repo" not in sys.path:
    sys.path.insert(0, "/opt/trn_rl_repo")

import numpy as np

B = 8
T = 2048
C = 1024
H = 64
P = 128
CB = C // P        # 8 contraction chunks of 128
TJ = T // 512      # 4 column strips of 512
NT = T // P        # 16 s-tiles
N_CORES = 8

PIECES = [4 * j + 4 for j in range(TJ)]   # ST pieces per strip: 4, 8, 12, 16
N_DUMMY = 5                               # PE warm-up matmuls (N=512)

_NC_CACHE = {}


def _build_nc():
    import concourse.bass as bass
    import concourse.mybir as mybir
    import concourse.tile as tile
    from concourse.bass import ts

    fp32 = mybir.dt.float32
    bf16 = mybir.dt.bfloat16
    u8 = mybir.dt.uint8
    EXP = mybir.ActivationFunctionType.Exp
    MULT = mybir.AluOpType.mult
    ADD = mybir.AluOpType.add

    nc = bass.Bass(target_bir_lowering=False, debug=False)
    xh_d = nc.declare_dram_parameter("xh", [TJ, P, CB, 512], bf16, isOutput=False)
    wh_d = nc.declare_dram_parameter("wh", [P, 2, CB, P], bf16, isOutput=False)
    ch_d = nc.declare_dram_parameter("ch", [P, 1024], u8, isOutput=False)
    out_d = nc.declare_dram_parameter("out", [T, H], fp32, isOutput=True)

    from contextlib import ExitStack

    with tile.TileContext(nc) as tc, ExitStack() as stk:
        pers = stk.enter_context(tc.tile_pool(name="pers", bufs=1))
        xt0a = pers.tile([P, CB // 2, 512], bf16, tag="xt0a", name="xt0a")
        xt0b = pers.tile([P, CB // 2, 512], bf16, tag="xt0b", name="xt0b")
        xts = [None] + [
            pers.tile([P, CB, 512], bf16, tag=f"xt{j}", name=f"xt{j}")
            for j in range(1, TJ)
        ]
        w_sb = pers.tile([P, 2, CB, P], bf16, tag="w_sb", name="w_sb")
        c_sb = pers.tile([P, 1024], u8, tag="c_sb", name="c_sb")
        # vq: vT @ 0:64 (for v transposes) + qT @ 64:128 (ST weights)
        vq = [pers.tile([P, 512], bf16, tag=f"vq{j}", name=f"vq{j}") for j in range(TJ)]
        # kk2: kT(scaled) @ 64:128 (ST rhs); top half unused
        kk2 = [pers.tile([P, 512], bf16, tag=f"kk2{j}", name=f"kk2{j}") for j in range(TJ)]
        pt_sb = [pers.tile([P, PIECES[j], 512], bf16, tag=f"pt{j}", name=f"pt{j}") for j in range(TJ)]
        vaug = [pers.tile([P, 4, H + 1], bf16, tag=f"va{j}", name=f"va{j}") for j in range(TJ)]
        oadd = [pers.tile([H + 1, 512], fp32, tag=f"oa{j}", name=f"oa{j}") for j in range(TJ)]
        o_sb = [pers.tile([P, 4, H], fp32, tag=f"o{j}", name=f"o{j}") for j in range(TJ)]
        rec = [pers.tile([P, 4], fp32, tag=f"rc{j}", name=f"rc{j}") for j in range(TJ)]
        scr_w = pers.tile([P, P], bf16, tag="scr_w", name="scr_w")
        scr_x = pers.tile([P, 512], bf16, tag="scr_x", name="scr_x")
        scr_rd = pers.tile([P, 1], fp32, tag="scr_rd", name="scr_rd")
        prim = pers.tile([P, 1], fp32, tag="prim", name="prim")
        prim_o = pers.tile([P, 1], fp32, tag="prim_o", name="prim_o")

        # constant views (shipped via DMA in ch)
        identb_lo = c_sb[0:H, 0:128].bitcast(bf16)        # [64, 64] @ 0:64
        identb128 = c_sb[:, 128:384].bitcast(bf16)        # [128, 128] (unused)
        tri = c_sb[:, 384:640].bitcast(bf16)              # [128, 128]
        ident65 = c_sb[0 : H + 1, 640:900].bitcast(fp32)  # [65, 65]

        # ---- input DMAs first: ONE ring (sync), strict strip order ----
        nc.sync.dma_start(w_sb[:], wh_d[:])
        nc.sync.dma_start(c_sb[:], ch_d[:])
        nc.sync.dma_start(xt0a[:], xh_d[0, :, 0 : CB // 2, :])
        nc.sync.dma_start(xt0b[:], xh_d[0, :, CB // 2 : CB, :])
        for j in range(1, TJ):
            nc.sync.dma_start(xts[j][:], xh_d[j])

        # ---- early gpsimd work: scratch memsets ----
        nc.gpsimd.memset(scr_w[:], 0.0)
        nc.gpsimd.memset(scr_x[:], 0.0)
        nc.gpsimd.memset(prim[:], 0.0)
        for j in range(TJ):
            nc.gpsimd.memset(vaug[j][:, :, H], 1.0)

        # ---- scalar primer: pull the exp table load into the DMA shadow ----
        nc.scalar.activation(prim_o[:], prim[:], EXP)

        def xsrc(j, cb):
            if j == 0:
                half = xt0a if cb < CB // 2 else xt0b
                return half[:, cb % (CB // 2), :]
            return xts[j][:, cb, :]

        # ---- PE warm-up dummies (read once by DVE so DCE keeps them) ----
        with tc.tile_pool(name="scrp", bufs=1, space="PSUM") as scrp:
            scr_ps = scrp.tile([P, 512], fp32, tag="scr", name="scr_ps")
            for k in range(N_DUMMY):
                nc.tensor.matmul(
                    scr_ps, scr_w[:], scr_x[:], start=(k == 0), stop=(k == N_DUMMY - 1)
                )
            nc.vector.tensor_copy(scr_rd[:], scr_ps[:, 0:1])

        with (
            tc.tile_pool(name="prjp", bufs=1, space="PSUM") as prjp,   # 1 bank
            tc.tile_pool(name="tpp", bufs=1, space="PSUM") as tpp,     # 1 bank
            tc.tile_pool(name="stp", bufs=2, space="PSUM") as stp,     # 4 banks
            tc.tile_pool(name="otp", bufs=1, space="PSUM") as otp,     # 2 banks
        ):
            ot_a = otp.tile([H + 1, 512], fp32, tag="ota", name="ot_a")
            ot_b = otp.tile([H + 1, 512], fp32, tag="otb", name="ot_b")

            def proj_steps(j):
                """Generator: one projection matmul per step (17 steps:
                8 P1 chunks, drain, 8 P2 chunks, drain, v transposes)."""
                p1 = prjp.tile([P, 512], fp32, tag="prj", name=f"p1_{j}")
                for cb in range(CB):
                    nc.tensor.matmul(
                        p1, w_sb[:, 0, cb, :], xsrc(j, cb),
                        start=(cb == 0), stop=(cb == CB - 1),
                    )
                    yield
                nc.vector.tensor_copy(vq[j][:], p1)
                p2 = prjp.tile([P, 512], fp32, tag="prj", name=f"p2_{j}")
                for cb in range(CB):
                    nc.tensor.matmul(
                        p2, w_sb[:, 1, cb, :], xsrc(j, cb),
                        start=(cb == 0), stop=(cb == CB - 1),
                    )
                    yield
                nc.vector.tensor_copy(kk2[j][H:P, :], p2[H:P, :])
                # v transposes: vq[j][0:64, block m] -> vaug[j][:, m, 0:64]
                vt = tpp.tile([P, 4, H], bf16, tag="tp", name=f"vt{j}")
                for m in range(4):
                    nc.tensor.transpose(vt[:, m, :], vq[j][0:H, ts(m, P)], identb_lo)
                    yield
                nc.vector.tensor_copy(vaug[j][:, :, 0:H], vt)

            def st_steps(j):
                """Generator: one ST group (two s-tiles + exp) per step."""
                for m in range(2 * j + 2):
                    iA, iB = 2 * m, 2 * m + 1
                    o = max(0, P * iA - 512 * j)
                    oB = max(0, P * iB - 512 * j)
                    s2 = stp.tile([P, 2, 512], fp32, tag="st", name=f"s{j}_{m}")
                    nc.tensor.matmul(
                        s2[:, 0, o:512],
                        vq[iA // 4][H:P, ts(iA % 4, P)],
                        kk2[j][H:P, o:512],
                        start=True, stop=True,
                    )
                    nc.tensor.matmul(
                        s2[:, 1, oB:512],
                        vq[iB // 4][H:P, ts(iB % 4, P)],
                        kk2[j][H:P, oB:512],
                        start=True, stop=True,
                    )
                    nc.scalar.activation(
                        pt_sb[j][:, 2 * m : 2 * m + 2, o:512],
                        s2[:, :, o:512],
                        EXP,
                    )
                    yield m

            def emit_tri(j):
                for i in range(4 * j, 4 * j + 4):
                    o = P * i - 512 * j
                    nc.vector.tensor_tensor(
                        pt_sb[j][:, i, o : o + P],
                        pt_sb[j][:, i, o : o + P],
                        tri, MULT,
                    )

            def pv_steps(j):
                """Generator: one PV chunk (row-tiled lockstep pair) per step."""
                n = PIECES[j]
                for i in range(n):
                    o = max(0, P * i - 512 * j)
                    nc.tensor.matmul(
                        ot_a[:, o:512],
                        vaug[i // 4][0:H, i % 4, :],
                        pt_sb[j][0:H, i, o:512],
                        start=(i == 0), stop=(i == n - 1),
                    )
                    nc.tensor.matmul(
                        ot_b[:, o:512],
                        vaug[i // 4][H:P, i % 4, :],
                        pt_sb[j][H:P, i, o:512],
                        start=(i == 0), stop=(i == n - 1),
                    )
                    yield

            def emit_epilogue(j):
                nc.vector.tensor_copy(oadd[j][:], ot_a[:, :])
                nc.vector.tensor_tensor(oadd[j][:], ot_b[:, :], oadd[j][:], ADD)
                orr = tpp.tile([P, 4, H + 1], fp32, tag="tp", name=f"or{j}")
                for m in range(4):
                    nc.tensor.transpose(orr[:, m, :], oadd[j][:, ts(m, P)], ident65)
                nc.vector.reciprocal(rec[j][:, :], orr[:, :, H])
                for m in range(4):
                    nc.vector.tensor_scalar_mul(
                        o_sb[j][:, m, :], orr[:, m, 0:H], rec[j][:, m : m + 1]
                    )
                nc.gpsimd.dma_start(
                    out_d[ts(j, 512), :].rearrange("(m p) d -> p m d", p=P),
                    o_sb[j][:],
                )

            def drain(gen, k=1):
                """Advance a generator k steps; False when exhausted."""
                for _ in range(k):
                    if gen is None:
                        return None
                    if next(gen, "END") == "END":
                        return None
                return gen

            # ---- pipeline ----
            # strip 0 projections stand alone (nothing to interleave with)
            g = proj_steps(0)
            while drain(g) is not None:
                pass
            for j in range(TJ):
                if j > 0:
                    emit_tri(j - 1)
                prj = proj_steps(j + 1) if j + 1 < TJ else None
                pv = pv_steps(j - 1) if j > 0 else None
                npairs = 2 * j + 2
                npv = PIECES[j - 1] if j > 0 else 0
                # per ST group: ~npv/npairs PV chunks + ~20/npairs proj steps
                pvd = prjd = 0
                for m in st_steps(j):
                    pvt = (npv * (m + 1)) // npairs
                    pv = drain(pv, pvt - pvd)
                    pvd = pvt
                    prt = (20 * (m + 1)) // npairs
                    prj = drain(prj, prt - prjd)
                    prjd = prt
                while pv is not None:
                    pv = drain(pv)
                while prj is not None:
                    prj = drain(prj)
                if j > 0:
                    emit_epilogue(j - 1)
            emit_tri(TJ - 1)
            pv = pv_steps(TJ - 1)
            while drain(pv) is not None:
                pass
            emit_epilogue(TJ - 1)

    return nc


def _split_multiwaits(nc):
    """Walrus codegen only supports one sync-wait command per instruction;
    hoist extra waits onto NoOps inserted just before (same engine queue,
    identical semantics since engines execute their queue in order)."""
    import concourse.mybir as mybir

    n = 0
    for fn in nc.m.functions:
        for block in fn.blocks:
            new_insts = []
            for inst in block.instructions:
                si = inst.sync_info
                if si is not None and si.on_wait and len(si.on_wait) > 1:
                    waits = list(si.on_wait)
                    for w in waits[:-1]:
                        n += 1
                        new_insts.append(
                            mybir.InstNoOp(
                                name=f"WH-{n}", engine=inst.engine, ins=[], outs=[],
                                sync_info=mybir.SyncInfo(on_wait=[w], on_update=[]),
                            )
                        )
                    si.on_wait = waits[-1:]
                new_insts.append(inst)
            block.instructions = new_insts
    return nc


def _get_nc():
    if "nc" not in _NC_CACHE:
        _NC_CACHE["nc"] = _split_multiwaits(_build_nc())
    return _NC_CACHE["nc"]


def _make_consts():
    import ml_dtypes

    bf16 = ml_dtypes.bfloat16
    ch = np.zeros((P, 1024), dtype=np.uint8)
    idb2 = np.zeros((P, H), dtype=bf16)
    idb2[0:H] = np.eye(H, dtype=bf16)
    idb2[H:P] = np.eye(H, dtype=bf16)
    ch[:, 0:128] = idb2.view(np.uint8)
    ch[:, 128:384] = np.eye(P, dtype=bf16).view(np.uint8)
    tri = np.triu(np.ones((P, P), dtype=np.float32)).astype(bf16)
    ch[:, 384:640] = tri.view(np.uint8)
    id65 = np.eye(H + 1, dtype=np.float32)
    ch[0 : H + 1, 640:900] = id65.view(np.uint8)
    return ch


def _make_in_maps(x, Wk, Wq, Wv):
    import ml_dtypes

    bf16 = ml_dtypes.bfloat16
    scale = 1.0 / np.sqrt(np.float32(C))
    w1 = np.concatenate([Wv, Wq], axis=0).T           # [C, 128] -> vT+qT
    w2 = np.concatenate([Wk * scale, Wk * scale], axis=0).T  # [C, 128] -> kT
    wh = np.ascontiguousarray(
        np.stack([w1, w2]).astype(bf16)
        .reshape(2, CB, P, P).transpose(2, 0, 1, 3)
    )
    ch = _make_consts()
    in_maps = []
    for b in range(B):
        xt = x[b].T.astype(bf16)  # [C, T]
        xhb = np.ascontiguousarray(
            xt.reshape(CB, P, TJ, 512).transpose(2, 1, 0, 3)
        )
        in_maps.append({"xh": xhb, "wh": wh, "ch": ch})
    return in_maps


def run(x, Wk, Wq, Wv, trace=False):
    from concourse.bass_utils import run_bass_kernel_spmd

    nc = _get_nc()
    in_maps = _make_in_maps(x, Wk, Wq, Wv)
    res = run_bass_kernel_spmd(nc, in_maps, core_ids=list(range(N_CORES)), trace=trace)
    out = np.stack([np.asarray(res.results[b]["out"]) for b in range(B)], axis=0)
    return out.astype(np.float32), res


def kernel(x, Wk, Wq, Wv):
    out, _ = run(x, Wk, Wq, Wv, trace=False)
    return out
